# revision 4
# baseline (speedup 1.0000x reference)
"""MQA attention kernel for Trainium2, 8-core SPMD (v2).

Problem: Q [2, 8, 2048, 64] fp32, K/V [2, 1, 2048, 64] fp32 (shared head).
out[b,h,q,:] = softmax(Q[b,h,q,:] @ K[b,0]^T / 8) @ V[b,0].

Sharding: 16 (b,h) pairs over 8 cores -> core c handles b = c//4,
heads 2*(c%4), 2*(c%4)+1 (both heads share one K/V slice).

v2 design (vs v1's PE-transpose staging + ACT-only exp):
  - Zero PE transposes. Q/K are cast to fp16 HBM scratch (SWDGE cast-DMA)
    and landed as QT/KT [128, S] via XBAR transpose-DMA chunks; the K
    scratch holds K twice so KT has K^T on both partition halves.
  - MM1 (scores^T): per k-tile, two concurrent row-group matmuls
    (contract=64; h0 on rows 0-63, h1 on 64-127) -> ps_s [128k, 2, 512q].
  - exp split across engines: ACT computes true exp for h0 (scale/bias
    folded); DVE computes Schraudolph exp for h1 with one tensor_scalar:
    int16(round(score*A + B)) bit-cast as fp16 ~= exp(score/8 - C).
    The shared -C shift cancels in softmax normalization.
  - MM2: out^T[80, 512] += V_aug[kt]^T @ P^T; V_aug cols = [V | ones |
    zero-pad to 80]. Column 64 accumulates the softmax denominator; the
    80-row pad makes the drain transpose-DMA legal (p%16==0) with no
    garbage reads.
  - Drain: copy out^T PSUM->SBUF fp16 (ACT for h0, DVE for h1), XBAR
    transpose-DMA to [512q, 80], DVE reciprocal of col 64, GPSIMD
    tensor_scalar normalize -> fp32, DMA out.
  - Main loop software-pipelined: MM1(kt+1) is issued before MM2(kt) so
    the PE never waits on exp; ps_s/ps_o double-buffered = 8 PSUM banks.
"""

import numpy as np

import concourse.bass as bass
import concourse.bacc as bacc
import concourse.mybir as mybir
import concourse.tile as tile
from concourse.bass_utils import run_bass_kernel_spmd

F32 = mybir.dt.float32
F16 = mybir.dt.float16
I16 = mybir.dt.int16

B, H, S, D = 2, 8, 2048, 64
HPC = 2            # heads per core
NCORES = 8
QB = 512           # query block (PSUM bank free-dim limit for fp32)
NQB = S // QB      # 4
KT_TILE = 128      # keys per k-tile (PE contract partition limit)
NKT = S // KT_TILE # 16
MO = 80            # MM2 output rows: 64 V + 1 denom + 15 zero pad (16|80)
SCALE = 1.0 / np.sqrt(np.float32(D))  # 0.125
C_SHIFT = 2.0      # exp(z - C_SHIFT): bounds p in fp16/int16; cancels in softmax

# Schraudolph fp16 exp: i16 = round(score*A2 + B2); bitcast fp16 ~= exp(score/8 - C)
A_FP16 = 1024.0 / np.log(2.0)              # 2^10 / ln 2
A2 = float(SCALE) * A_FP16                 # folded score scale
B2 = 15.0 * 1024.0 - C_SHIFT * A_FP16      # exponent bias - shift


def build_nc():
    nc = bacc.Bacc(None)
    Qd = nc.declare_dram_parameter("q", [HPC, S, D], F32, isOutput=False)
    Kd = nc.declare_dram_parameter("k", [S, D], F32, isOutput=False)
    Vd = nc.declare_dram_parameter("v", [S, D], F32, isOutput=False)
    Od = nc.declare_dram_parameter("o", [HPC, S, D], F32, isOutput=True)

    with tile.TileContext(nc) as tc:
        with (
            tc.tile_pool(name="scratch", bufs=1, space="DRAM") as dramp,
            tc.tile_pool(name="const", bufs=1) as constp,
            tc.tile_pool(name="qk", bufs=1) as qkp,
            tc.tile_pool(name="vt", bufs=1) as vp,
            tc.tile_pool(name="pt", bufs=3) as ptp,
            tc.tile_pool(name="otF", bufs=2) as otFp,
            tc.tile_pool(name="otT", bufs=2) as otTp,
            tc.tile_pool(name="rec", bufs=2) as recp,
            tc.tile_pool(name="outsb", bufs=2) as outp,
            tc.tile_pool(name="psS", bufs=2, space="PSUM") as psSp,
            tc.tile_pool(name="psO", bufs=2, space="PSUM") as psOp,
        ):
            # Prime the exp table load so the ~2.7us ACT_TABLE_LOAD overlaps
            # the input DMA phase instead of stalling the first real exp.
            dummy = constp.tile([128, 16], F32)
            nc.vector.memset(dummy[:], 0.0)
            nc.scalar.activation(dummy[:], dummy[:], mybir.ActivationFunctionType.Exp)

            # per-partition bias AP for the ACT exp (const-AP registry only
            # carries pre-registered values)
            bias_ap = constp.tile([128, 1], F32)
            nc.vector.memset(bias_ap[:], -float(C_SHIFT))

            # ---- input staging: cast-DMA to fp16 HBM scratch, then XBAR
            # transpose-DMA into SBUF. K lands first (all k-tiles are needed
            # by qb0); Q is chunked per qb so qb0 can start early. ----
            Ksc = dramp.tile([S, 128], F16, name="Ksc")
            nc.gpsimd.dma_start(out=Ksc[:, 0:D], in_=Kd.ap())
            nc.gpsimd.dma_start(out=Ksc[:, D : 2 * D], in_=Kd.ap())

            Qsc = dramp.tile([S, 128], F16, name="Qsc")
            for qb in range(NQB):
                qs = slice(qb * QB, (qb + 1) * QB)
                nc.gpsimd.dma_start(
                    out=Qsc[qs, :].rearrange("s (h d) -> s h d", h=HPC),
                    in_=Qd.ap()[:, qs, :].rearrange("h s d -> s h d"),
                )

            # V tiles [128k, kt, 80] fp16: cols 0-63 = V, col 64 = 1.0
            # (denominator accumulator), cols 65-79 = 0 (transpose pad).
            Vt = vp.tile([128, NKT, MO], F16)
            nc.vector.memset(Vt[:, :, D + 1 : MO], 0.0)
            nc.vector.memset(Vt[:, :, D : D + 1], 1.0)
            nc.gpsimd.dma_start(
                out=Vt[:, :, 0:D],
                in_=Vd.ap().rearrange("(t p) d -> p t d", p=128),
            )

            # KT/QT [128, S] fp16: K^T replicated on both partition halves;
            # Q0^T on partitions 0-63, Q1^T on 64-127.
            KT = qkp.tile([128, S], F16, name="KT")
            QT = qkp.tile([128, S], F16, name="QT")
            for qb in range(NQB):
                qs = slice(qb * QB, (qb + 1) * QB)
                nc.sync.dma_start(out=KT[:, qs], in_=Ksc[qs, :], transpose=True)
            for qb in range(NQB):
                qs = slice(qb * QB, (qb + 1) * QB)
                nc.sync.dma_start(out=QT[:, qs], in_=Qsc[qs, :], transpose=True)

            # ---- main loop ----
            for qb in range(NQB):
                qs = slice(qb * QB, (qb + 1) * QB)
                ps_o = [psOp.tile([MO, QB], F32, name=f"psO{h}") for h in range(HPC)]
                pending = None  # software pipeline: MM2 trails MM1 by one kt
                for kt in range(NKT):
                    ks = slice(kt * KT_TILE, (kt + 1) * KT_TILE)
                    ps_s = psSp.tile([128, HPC, QB], F32)
                    for h in range(HPC):
                        nc.tensor.matmul(
                            ps_s[:, h, :],
                            lhsT=KT[64 * h : 64 * (h + 1), ks],
                            rhs=QT[64 * h : 64 * (h + 1), qs],
                            start=True,
                            stop=True,
                        )
                    if pending is not None:
                        pkt, p0, p1 = pending
                        for h, rhs in ((0, p0[:]), (1, p1[:].bitcast(F16))):
                            nc.tensor.matmul(
                                ps_o[h][:],
                                lhsT=Vt[:, pkt, :],
                                rhs=rhs,
                                start=(pkt == 0),
                                stop=False,
                            )
                    # exp: ACT (true) for h0, DVE (Schraudolph) for h1
                    pt0 = ptp.tile([128, QB], F16, name="pt0")
                    nc.scalar.activation(
                        pt0[:],
                        ps_s[:, 0, :],
                        mybir.ActivationFunctionType.Exp,
                        scale=float(SCALE),
                        bias=bias_ap[:],
                    )
                    pt1 = ptp.tile([128, QB], I16, name="pt1")
                    nc.vector.tensor_scalar(
                        pt1[:],
                        ps_s[:, 1, :],
                        float(A2),
                        float(B2),
                        op0=mybir.AluOpType.mult,
                        op1=mybir.AluOpType.add,
                    )
                    pending = (kt, pt0, pt1)
                pkt, p0, p1 = pending
                for h, rhs in ((0, p0[:]), (1, p1[:].bitcast(F16))):
                    nc.tensor.matmul(
                        ps_o[h][:],
                        lhsT=Vt[:, pkt, :],
                        rhs=rhs,
                        start=False,
                        stop=True,
                    )

                # ---- drain: fp16 copy, transpose-DMA, normalize, store ----
                for h in range(HPC):
                    otF = otFp.tile([MO, QB], F16)
                    if h == 0:
                        nc.scalar.copy(otF[:], ps_o[h][:])
                    else:
                        nc.vector.tensor_copy(otF[:], ps_o[h][:])
                    otT = otTp.tile([128, QB // 128, MO], F16)
                    for j in range(QB // 128):
                        nc.sync.dma_start(
                            out=otT[:, j, :],
                            in_=otF[:, j * 128 : (j + 1) * 128],
                            transpose=True,
                        )
                    rec = recp.tile([128, QB // 128, 1], F32)
                    nc.vector.reciprocal(rec[:], otT[:, :, D : D + 1])
                    outsb = outp.tile([128, QB // 128, D], F32)
                    for j in range(QB // 128):
                        nc.gpsimd.tensor_scalar_mul(
                            outsb[:, j, :], otT[:, j, 0:D], rec[:, j, :]
                        )
                    nc.sync.dma_start(
                        out=Od.ap()[h, qs, :].rearrange("(j p) d -> p j d", p=128),
                        in_=outsb[:],
                    )
    nc.compile()
    return nc


_CACHED = {}


def _get_nc():
    if "nc" not in _CACHED:
        _CACHED["nc"] = build_nc()
    return _CACHED["nc"]


def _shard(Q, K, V):
    in_maps = []
    for c in range(NCORES):
        b = c // 4
        h0 = (c % 4) * HPC
        in_maps.append(
            {
                "q": np.ascontiguousarray(np.asarray(Q, np.float32)[b, h0 : h0 + HPC]),
                "k": np.ascontiguousarray(np.asarray(K, np.float32)[b, 0]),
                "v": np.ascontiguousarray(np.asarray(V, np.float32)[b, 0]),
            }
        )
    return in_maps


def kernel(Q, K, V, trace=False):
    nc = _get_nc()
    res = run_bass_kernel_spmd(nc, _shard(Q, K, V), list(range(NCORES)), trace=trace)
    _CACHED["last_result"] = res
    O = np.empty((B, H, S, D), np.float32)
    for c, r in enumerate(res.results):
        b = c // 4
        h0 = (c % 4) * HPC
        O[b, h0 : h0 + HPC] = r["o"]
    return O


# revision 6
# speedup vs baseline: 1.4087x; 1.4087x over previous
"""MQA attention kernel for Trainium2, 8-core SPMD (v3).

Problem: Q [2, 8, 2048, 64] fp32, K/V [2, 1, 2048, 64] fp32 (shared head).
out[b,h,q,:] = softmax(Q[b,h,q,:] @ K[b,0]^T / 8) @ V[b,0].

Sharding: 16 (b,h) pairs over 8 cores -> core c handles b = c//4,
heads 2*(c%4), 2*(c%4)+1 (both heads share one K/V slice).

Design:
  - Zero PE transposes. Q/K are cast to an fp16 HBM scratch (SWDGE
    cast-DMA) laid out [S, (h|rep, d)], then landed as QT/KT [128, S] via
    XBAR transpose-DMA (K: 4 key-chunks; Q: 4 query-chunks so qb0 starts
    early). The K scratch holds K twice so KT has K^T on both halves.
  - MM1 (scores^T): per k-tile, two concurrent row-group matmuls
    (contract=64; h0 rows 0-63, h1 rows 64-127) -> ps_s [128k, 2, 512q].
  - exp split: ACT true exp for h0 (scale/bias folded); DVE Schraudolph
    exp for h1 in one tensor_scalar: int16(round(score*A+B)) bitcast fp16
    ~= exp(score/8 - C); the -C shift cancels in softmax.
  - MM2: out^T[80, 512] += V_aug[kt]^T @ P^T; V_aug = [V | ones | 0-pad].
    Col 64 accumulates the softmax denominator; 80 rows make the drain
    transpose legal (p%16==0) with no garbage.
  - Drain (per qb, h): PSUM->SBUF fp16 copy (ACT h0 / DVE h1), ONE merged
    XBAR transpose-DMA [80,512]->[128,4,80], DVE reciprocal of col 64,
    GPSIMD per-chunk tensor_scalar normalize -> fp32, DMA out. All drain
    ops are EMITTED INTERLEAVED into the next qb's kt loop so no engine
    FIFO stalls and the PE stays dense (HAM stays warm).
  - Main loop software-pipelined: MM1(kt+1) issues before MM2(kt); ps_s
    and ps_o double-buffered = exactly 8 PSUM banks.
"""

import numpy as np

import concourse.bass as bass
import concourse.bacc as bacc
import concourse.mybir as mybir
import concourse.tile as tile
from concourse.bass_utils import run_bass_kernel_spmd

F32 = mybir.dt.float32
F16 = mybir.dt.float16
I16 = mybir.dt.int16

B, H, S, D = 2, 8, 2048, 64
HPC = 2            # heads per core
NCORES = 8
QB = 512           # query block (PSUM bank free-dim limit for fp32)
NQB = S // QB      # 4
KT_TILE = 128      # keys per k-tile (PE contract partition limit)
NKT = S // KT_TILE # 16
MO = 80            # MM2 output rows: 64 V + 1 denom + 15 zero pad (16|80)
SCALE = 1.0 / np.sqrt(np.float32(D))  # 0.125
C_SHIFT = 2.0      # exp(z - C_SHIFT): bounds p in fp16/int16; cancels in softmax

# Schraudolph fp16 exp: i16 = round(score*A2 + B2); bitcast fp16 ~= exp(score/8 - C)
A_FP16 = 1024.0 / np.log(2.0)              # 2^10 / ln 2
A2 = float(SCALE) * A_FP16                 # folded score scale
B2 = 15.0 * 1024.0 - C_SHIFT * A_FP16      # exponent bias - shift


def build_nc():
    nc = bacc.Bacc(None)
    Qd = nc.declare_dram_parameter("q", [HPC, S, D], F32, isOutput=False)
    Kd = nc.declare_dram_parameter("k", [S, D], F32, isOutput=False)
    Vd = nc.declare_dram_parameter("v", [S, D], F32, isOutput=False)
    Od = nc.declare_dram_parameter("o", [HPC, S, D], F32, isOutput=True)

    with tile.TileContext(nc) as tc:
        with (
            tc.tile_pool(name="scratch", bufs=1, space="DRAM") as dramp,
            tc.tile_pool(name="const", bufs=1) as constp,
            tc.tile_pool(name="qk", bufs=1) as qkp,
            tc.tile_pool(name="vt", bufs=1) as vp,
            tc.tile_pool(name="pt", bufs=4) as ptp,
            tc.tile_pool(name="otF", bufs=2) as otFp,
            tc.tile_pool(name="otT", bufs=2) as otTp,
            tc.tile_pool(name="rec", bufs=2) as recp,
            tc.tile_pool(name="outsb", bufs=2) as outp,
            tc.tile_pool(name="psS", bufs=2, space="PSUM") as psSp,
            tc.tile_pool(name="psO", bufs=2, space="PSUM") as psOp,
        ):
            # Prime the exp table load so the ~2.7us ACT_TABLE_LOAD overlaps
            # the input DMA phase instead of stalling the first real exp.
            dummy = constp.tile([128, 16], F32)
            nc.vector.memset(dummy[:], 0.0)
            nc.scalar.activation(dummy[:], dummy[:], mybir.ActivationFunctionType.Exp)

            # per-partition bias AP for the ACT exp (const-AP registry only
            # carries pre-registered values)
            bias_ap = constp.tile([128, 1], F32)
            nc.vector.memset(bias_ap[:], -float(C_SHIFT))

            # ---- input staging: cast-DMA to fp16 HBM scratch, then XBAR
            # transpose-DMA chunks into SBUF ----
            Ksc = dramp.tile([S, 128], F16, name="Ksc")
            nc.gpsimd.dma_start(out=Ksc[:, 0:D], in_=Kd.ap())
            nc.gpsimd.dma_start(out=Ksc[:, D : 2 * D], in_=Kd.ap())
            Qsc = dramp.tile([S, 128], F16, name="Qsc")
            nc.gpsimd.dma_start(
                out=Qsc[:].rearrange("s (h d) -> s h d", h=HPC),
                in_=Qd.ap().rearrange("h s d -> s h d"),
            )

            # V tiles [128k, kt, 80] fp16: cols 0-63 = V, col 64 = 1.0
            # (denominator accumulator), cols 65-79 = 0 (transpose pad).
            Vt = vp.tile([128, NKT, MO], F16)
            nc.vector.memset(Vt[:, :, D + 1 : MO], 0.0)
            nc.vector.memset(Vt[:, :, D : D + 1], 1.0)
            nc.gpsimd.dma_start(
                out=Vt[:, :, 0:D],
                in_=Vd.ap().rearrange("(t p) d -> p t d", p=128),
            )

            # KT/QT [128, S] fp16: K^T replicated on both partition halves;
            # Q0^T on partitions 0-63, Q1^T on 64-127. Chunked transposes so
            # the first chunks land early.
            KT = qkp.tile([128, S], F16, name="KT")
            QT = qkp.tile([128, S], F16, name="QT")
            for c in range(NQB):
                cs = slice(c * QB, (c + 1) * QB)
                nc.sync.dma_start(out=KT[:, cs], in_=Ksc[cs, :], transpose=True)
            for c in range(NQB):
                cs = slice(c * QB, (c + 1) * QB)
                nc.sync.dma_start(out=QT[:, cs], in_=Qsc[cs, :], transpose=True)

            # ---- main loop; the previous qb's drain is emitted interleaved
            # into this qb's kt loop (stage list below) ----
            def drain_stages(qb, ps_o):
                qs = slice(qb * QB, (qb + 1) * QB)
                st = {"otF": [None, None], "otT": [None, None], "rec": [None, None]}

                def s_copy0():
                    st["otF"][0] = otFp.tile([MO, QB], F16, name="otF0")
                    nc.scalar.copy(st["otF"][0][:], ps_o[0][:])

                def s_copy1():
                    st["otF"][1] = otFp.tile([MO, QB], F16, name="otF1")
                    nc.vector.tensor_copy(st["otF"][1][:], ps_o[1][:])

                def s_transp(h):
                    def f():
                        st["otT"][h] = otTp.tile(
                            [128, QB // 128, MO], F16, name=f"otT{h}"
                        )
                        nc.sync.dma_start(
                            out=st["otT"][h][:], in_=st["otF"][h][:], transpose=True
                        )
                    return f

                def s_norm(h):
                    def f():
                        otT = st["otT"][h]
                        rec = recp.tile([128, QB // 128, 1], F32)
                        nc.vector.reciprocal(rec[:], otT[:, :, D : D + 1])
                        st["rec"][h] = rec
                        outsb = outp.tile([128, QB // 128, D], F32)
                        for j in range(QB // 128):
                            nc.gpsimd.tensor_scalar_mul(
                                outsb[:, j, :], otT[:, j, 0:D], rec[:, j, :]
                            )
                        nc.sync.dma_start(
                            out=Od.ap()[h, qs, :].rearrange("(j p) d -> p j d", p=128),
                            in_=outsb[:],
                        )
                    return f

                return [s_copy0, s_copy1, s_transp(0), s_transp(1), s_norm(0), s_norm(1)]

            pending_drain = []
            for qb in range(NQB):
                qs = slice(qb * QB, (qb + 1) * QB)
                ps_o = [psOp.tile([MO, QB], F32, name=f"psO{h}") for h in range(HPC)]
                pending = None  # software pipeline: MM2 trails MM1 by one kt
                for kt in range(NKT):
                    ks = slice(kt * KT_TILE, (kt + 1) * KT_TILE)
                    ps_s = psSp.tile([128, HPC, QB], F32)
                    for h in range(HPC):
                        nc.tensor.matmul(
                            ps_s[:, h, :],
                            lhsT=KT[64 * h : 64 * (h + 1), ks],
                            rhs=QT[64 * h : 64 * (h + 1), qs],
                            start=True,
                            stop=True,
                        )
                    if pending is not None:
                        pkt, p0, p1 = pending
                        for h, rhs in ((0, p0[:]), (1, p1[:].bitcast(F16))):
                            nc.tensor.matmul(
                                ps_o[h][:],
                                lhsT=Vt[:, pkt, :],
                                rhs=rhs,
                                start=(pkt == 0),
                                stop=False,
                            )
                    # previous qb's drain, spread one stage per kt step
                    if pending_drain and kt >= 1:
                        pending_drain.pop(0)()
                    # exp: ACT (true) for h0, DVE (Schraudolph) for h1
                    pt0 = ptp.tile([128, QB], F16, name="pt0")
                    nc.scalar.activation(
                        pt0[:],
                        ps_s[:, 0, :],
                        mybir.ActivationFunctionType.Exp,
                        scale=float(SCALE),
                        bias=bias_ap[:],
                    )
                    pt1 = ptp.tile([128, QB], I16, name="pt1")
                    nc.vector.tensor_scalar(
                        pt1[:],
                        ps_s[:, 1, :],
                        float(A2),
                        float(B2),
                        op0=mybir.AluOpType.mult,
                        op1=mybir.AluOpType.add,
                    )
                    pending = (kt, pt0, pt1)
                pkt, p0, p1 = pending
                for h, rhs in ((0, p0[:]), (1, p1[:].bitcast(F16))):
                    nc.tensor.matmul(
                        ps_o[h][:],
                        lhsT=Vt[:, pkt, :],
                        rhs=rhs,
                        start=False,
                        stop=True,
                    )
                assert not pending_drain
                pending_drain = drain_stages(qb, ps_o)
            for f in pending_drain:
                f()
    nc.compile()
    return nc


_CACHED = {}


def _get_nc():
    if "nc" not in _CACHED:
        _CACHED["nc"] = build_nc()
    return _CACHED["nc"]


def _shard(Q, K, V):
    in_maps = []
    for c in range(NCORES):
        b = c // 4
        h0 = (c % 4) * HPC
        in_maps.append(
            {
                "q": np.ascontiguousarray(np.asarray(Q, np.float32)[b, h0 : h0 + HPC]),
                "k": np.ascontiguousarray(np.asarray(K, np.float32)[b, 0]),
                "v": np.ascontiguousarray(np.asarray(V, np.float32)[b, 0]),
            }
        )
    return in_maps


def kernel(Q, K, V, trace=False):
    nc = _get_nc()
    res = run_bass_kernel_spmd(nc, _shard(Q, K, V), list(range(NCORES)), trace=trace)
    _CACHED["last_result"] = res
    O = np.empty((B, H, S, D), np.float32)
    for c, r in enumerate(res.results):
        b = c // 4
        h0 = (c % 4) * HPC
        O[b, h0 : h0 + HPC] = r["o"]
    return O


# revision 14
# speedup vs baseline: 1.5866x; 1.1263x over previous
"""MQA attention kernel for Trainium2, 8-core SPMD (v4).

Problem: Q [2, 8, 2048, 64] fp32, K/V [2, 1, 2048, 64] fp32 (shared head).
out[b,h,q,:] = softmax(Q[b,h,q,:] @ K[b,0]^T / 8) @ V[b,0].

Sharding: 16 (b,h) pairs over 8 cores -> core c handles b = c//4,
heads 2*(c%4), 2*(c%4)+1 (both heads share one K/V slice).

Design:
  - Zero PE transposes, zero SWDGE, zero DRAM scratch. Q/K/V load fp32
    via HWDGE straight into SBUF, are cast to fp16 on ACT/DVE (idle in
    the prologue), and QT/KT [128, S] are produced by merged SBUF->SBUF
    XBAR transpose-DMAs ([128, F] -> [P, F/P*? chunks] 3D-out form). K^T
    is replicated to both partition halves with one SBUF copy DMA.
  - MM1 (scores^T): per k-tile, two concurrent row-group matmuls
    (contract=64; h0 rows 0-63, h1 rows 64-127) -> ps_s [128k, 2, 512q].
  - exp split: ACT true exp for h0; DVE Schraudolph exp for h1 in one
    tensor_scalar: int16(round(score*A+B)) bitcast fp16 ~= exp(score/8-C)
    (the -C shift cancels in softmax).
  - MM2: out^T[128, 512] += V_aug[kt]^T @ P^T; V_aug = [V | ones | 0-pad
    to 128 cols] (128 weight cols enables fast-weight-load; PSUM cost is
    the same one bank). Col 64 accumulates the softmax denominator.
  - Drain (per qb, h): PSUM->SBUF fp16 copy of rows 0:80 (ACT h0 / DVE
    h1), ONE merged XBAR transpose [80,512]->[128,4,80], DVE reciprocal
    of col 64, GPSIMD tensor_scalar normalize -> fp32, DMA out. Drain is
    EMITTED INTERLEAVED into the next qb's kt loop (no FIFO stalls).
  - Main loop software-pipelined with MM2 trailing MM1 by TWO k-tiles so
    the PE FIFO never waits on exp. ps_s/ps_o double-buffered = 8 banks.
  - PE warmup matmuls during the prologue so HAM is at K=8/8 when the
    main loop starts.
"""

import numpy as np

import concourse.bass as bass
import concourse.bacc as bacc
import concourse.mybir as mybir
import concourse.tile as tile
from concourse.bass_utils import run_bass_kernel_spmd

F32 = mybir.dt.float32
F16 = mybir.dt.float16
I16 = mybir.dt.int16

B, H, S, D = 2, 8, 2048, 64
HPC = 2            # heads per core
NCORES = 8
QB = 512           # query block (PSUM bank free-dim limit for fp32)
NQB = S // QB      # 4
KT_TILE = 128      # keys per k-tile (PE contract partition limit)
NKT = S // KT_TILE # 16
MO = 80            # drained rows: 64 V + 1 denom + 15 pad (16 | 80)
VW = 128           # V_aug weight cols (128 -> FWL-eligible LDWEIGHTS)
SCALE = 1.0 / np.sqrt(np.float32(D))  # 0.125
C_SHIFT = 2.0      # exp(z - C_SHIFT): bounds p in fp16/int16; cancels in softmax

# Schraudolph fp16 exp: i16 = round(score*A2 + B2); bitcast fp16 ~= exp(score/8 - C)
A_FP16 = 1024.0 / np.log(2.0)              # 2^10 / ln 2
A2 = float(SCALE) * A_FP16                 # folded score scale
B2 = 15.0 * 1024.0 - C_SHIFT * A_FP16      # exponent bias - shift
MM2_SKEW = 2


def build_nc():
    nc = bacc.Bacc(None)
    Qd = nc.declare_dram_parameter("q", [HPC, S, D], F32, isOutput=False)
    Kd = nc.declare_dram_parameter("k", [S, D], F32, isOutput=False)
    Vd = nc.declare_dram_parameter("v", [S, D], F32, isOutput=False)
    Od = nc.declare_dram_parameter("o", [HPC, S, D], F32, isOutput=True)

    with tile.TileContext(nc) as tc:
        with (
            tc.tile_pool(name="const", bufs=1) as constp,
            tc.tile_pool(name="stage", bufs=1) as stp,
            tc.tile_pool(name="qk", bufs=1) as qkp,
            tc.tile_pool(name="vt", bufs=1) as vp,
            tc.tile_pool(name="pt", bufs=6) as ptp,
            tc.tile_pool(name="otF", bufs=2) as otFp,
            tc.tile_pool(name="otT", bufs=2) as otTp,
            tc.tile_pool(name="rec", bufs=2) as recp,
            tc.tile_pool(name="outsb", bufs=2) as outp,
            tc.tile_pool(name="psS", bufs=2, space="PSUM") as psSp,
            tc.tile_pool(name="psO", bufs=2, space="PSUM") as psOp,
        ):
            # Prime the exp table load so the ~2.7us ACT_TABLE_LOAD overlaps
            # the input DMA phase instead of stalling the first real exp.
            dummy = constp.tile([128, 64], F16)
            nc.vector.memset(dummy[:], 0.0)
            dummy32 = constp.tile([128, 16], F32)
            nc.vector.memset(dummy32[:], 0.0)
            nc.scalar.activation(
                dummy32[:], dummy32[:], mybir.ActivationFunctionType.Exp
            )

            # per-partition bias AP for the ACT exp (const-AP registry only
            # carries pre-registered values)
            bias_ap = constp.tile([128, 1], F32)
            nc.vector.memset(bias_ap[:], -float(C_SHIFT))

            # ---- input staging: HWDGE fp32 loads, on-chip fp16 casts,
            # merged SBUF->SBUF XBAR transposes ----
            Kn = stp.tile([128, NKT, D], F32, name="Kn")
            nc.sync.dma_start(
                out=Kn[:], in_=Kd.ap().rearrange("(t p) d -> p t d", p=128)
            )
            # K cast duplicated into both 64-col halves: the XBAR consumes
            # input columns in 128-wide tiles, and the duplicate makes the
            # transposed KT carry K^T on both partition halves for free.
            Kh = stp.tile([128, NKT, 2, D], F16, name="Kh")
            nc.vector.tensor_copy(Kh[:, :, 0, :], Kn[:])
            nc.vector.tensor_copy(Kh[:, :, 1, :], Kn[:])

            Qn = stp.tile([128, NKT, HPC, D], F32, name="Qn")
            for h in range(HPC):
                nc.sync.dma_start(
                    out=Qn[:, :, h, :],
                    in_=Qd.ap()[h].rearrange("(t p) d -> p t d", p=128),
                )
            Qh = stp.tile([128, NKT, HPC, D], F16, name="Qh")
            # chunked casts (split ACT/DVE) so QT chunks can land early
            for c in range(NQB):
                cs = slice(c * (NKT // NQB), (c + 1) * (NKT // NQB))
                eng = nc.scalar if c % 2 == 0 else nc.vector
                if eng is nc.scalar:
                    nc.scalar.copy(Qh[:, cs, :, :], Qn[:, cs, :, :])
                else:
                    nc.vector.tensor_copy(Qh[:, cs, :, :], Qn[:, cs, :, :])

            Vn = stp.tile([128, NKT, D], F32, name="Vn")
            nc.sync.dma_start(
                out=Vn[:], in_=Vd.ap().rearrange("(t p) d -> p t d", p=128)
            )
            # V_aug [128k, kt, 128] fp16: cols 0-63 = V, col 64 = 1.0
            # (denominator), cols 65-127 = 0 (drain pad + FWL-width pad).
            Vt = vp.tile([128, NKT, VW], F16)
            nc.vector.memset(Vt[:, :, D + 1 : VW], 0.0)
            nc.vector.memset(Vt[:, :, D : D + 1], 1.0)
            nc.scalar.copy(Vt[:, :, 0:D], Vn[:])

            # PE warmup: keep the PE busy through the staging phase so HAM
            # is un-throttled (K=8/8) when the main loop starts. Scribbles
            # into qb0's out accumulator, which MM2 kt=0 (start=True) clears.
            ps_o0 = [psOp.tile([128, QB], F32, name=f"psO{h}") for h in range(HPC)]
            for _ in range(72):
                nc.tensor.matmul(
                    ps_o0[0][0:64, 0:64],
                    lhsT=dummy[:, 0:64],
                    rhs=dummy[:],
                    start=True,
                    stop=True,
                )

            # KT [128, S]: one merged XBAR transpose; the column-duplicated
            # input lands K^T on both partition halves directly.
            KT = qkp.tile([128, NKT, 128], F16, name="KT")
            nc.sync.dma_start(
                out=KT[:],
                in_=Kh[:].rearrange("p t r d -> p (t r d)"),
                transpose=True,
            )
            # QT [128, S]: chunked merged transposes (4 k-tiles each) so qb0
            # can start while later chunks stream in.
            QT = qkp.tile([128, NKT, 128], F16, name="QT")
            for c in range(NQB):
                cs = slice(c * (NKT // NQB), (c + 1) * (NKT // NQB))
                nc.sync.dma_start(
                    out=QT[:, cs, :],
                    in_=Qh[:, cs, :, :].rearrange("p t h d -> p (t h d)"),
                    transpose=True,
                )

            def QTs(h, qb):
                # [64, 512] moving operand for head h, query block qb
                return (
                    QT[64 * h : 64 * (h + 1), :, :]
                    .rearrange("p t d -> p (t d)")[:, qb * QB : (qb + 1) * QB]
                )

            def KTs(h, kt):
                return KT[64 * h : 64 * (h + 1), kt, :]

            # ---- main loop; the previous qb's drain is emitted interleaved
            # into this qb's kt loop ----
            def drain_stages(qb, ps_o):
                qs = slice(qb * QB, (qb + 1) * QB)
                st = {"otF": [None, None], "otT": [None, None]}

                def s_copy0():
                    st["otF"][0] = otFp.tile([MO, QB], F16, name="otF0")
                    nc.scalar.copy(st["otF"][0][:], ps_o[0][0:MO, :])

                def s_copy1():
                    st["otF"][1] = otFp.tile([MO, QB], F16, name="otF1")
                    nc.vector.tensor_copy(st["otF"][1][:], ps_o[1][0:MO, :])

                def s_transp(h):
                    def f():
                        st["otT"][h] = otTp.tile(
                            [128, QB // 128, MO], F16, name=f"otT{h}"
                        )
                        nc.sync.dma_start(
                            out=st["otT"][h][:], in_=st["otF"][h][:], transpose=True
                        )
                    return f

                def s_norm(h):
                    def f():
                        otT = st["otT"][h]
                        rec = recp.tile([128, QB // 128, 1], F32)
                        nc.vector.reciprocal(rec[:], otT[:, :, D : D + 1])
                        outsb = outp.tile([128, QB // 128, D], F32)
                        for j in range(QB // 128):
                            nc.gpsimd.tensor_scalar_mul(
                                outsb[:, j, :], otT[:, j, 0:D], rec[:, j, :]
                            )
                        nc.sync.dma_start(
                            out=Od.ap()[h, qs, :].rearrange("(j p) d -> p j d", p=128),
                            in_=outsb[:],
                        )
                    return f

                return [s_copy0, s_copy1, s_transp(0), s_transp(1), s_norm(0), s_norm(1)]

            pending_drain = []
            for qb in range(NQB):
                ps_o = (
                    ps_o0
                    if qb == 0
                    else [psOp.tile([128, QB], F32, name=f"psO{h}") for h in range(HPC)]
                )
                pend = []  # software pipeline: MM2 trails MM1 by MM2_SKEW kts
                for kt in range(NKT + MM2_SKEW + 1):
                    if kt < NKT:
                        ps_s = psSp.tile([128, HPC, QB], F32)
                        for h in range(HPC):
                            nc.tensor.matmul(
                                ps_s[:, h, :],
                                lhsT=KTs(h, kt),
                                rhs=QTs(h, qb),
                                start=True,
                                stop=True,
                            )
                    if len(pend) > (MM2_SKEW if kt < NKT else 0):
                        pkt, p0, p1 = pend.pop(0)
                        for h, rhs in ((0, p0[:]), (1, p1[:].bitcast(F16))):
                            nc.tensor.matmul(
                                ps_o[h][:],
                                lhsT=Vt[:, pkt, :],
                                rhs=rhs,
                                start=(pkt == 0),
                                stop=(pkt == NKT - 1),
                            )
                    if pending_drain and kt >= 1:
                        pending_drain.pop(0)()
                    if kt < NKT:
                        # exp: ACT (true) for h0, DVE (Schraudolph) for h1
                        pt0 = ptp.tile([128, QB], F16, name="pt0")
                        nc.scalar.activation(
                            pt0[:],
                            ps_s[:, 0, :],
                            mybir.ActivationFunctionType.Exp,
                            scale=float(SCALE),
                            bias=bias_ap[:],
                        )
                        pt1 = ptp.tile([128, QB], I16, name="pt1")
                        nc.vector.tensor_scalar(
                            pt1[:],
                            ps_s[:, 1, :],
                            float(A2),
                            float(B2),
                            op0=mybir.AluOpType.mult,
                            op1=mybir.AluOpType.add,
                        )
                        pend.append((kt, pt0, pt1))
                assert not pend and not pending_drain
                pending_drain = drain_stages(qb, ps_o)
            for f in pending_drain:
                f()
    nc.compile()
    return nc


_CACHED = {}


def _get_nc():
    if "nc" not in _CACHED:
        _CACHED["nc"] = build_nc()
    return _CACHED["nc"]


def _shard(Q, K, V):
    in_maps = []
    for c in range(NCORES):
        b = c // 4
        h0 = (c % 4) * HPC
        in_maps.append(
            {
                "q": np.ascontiguousarray(np.asarray(Q, np.float32)[b, h0 : h0 + HPC]),
                "k": np.ascontiguousarray(np.asarray(K, np.float32)[b, 0]),
                "v": np.ascontiguousarray(np.asarray(V, np.float32)[b, 0]),
            }
        )
    return in_maps


def kernel(Q, K, V, trace=False):
    nc = _get_nc()
    res = run_bass_kernel_spmd(nc, _shard(Q, K, V), list(range(NCORES)), trace=trace)
    _CACHED["last_result"] = res
    O = np.empty((B, H, S, D), np.float32)
    for c, r in enumerate(res.results):
        b = c // 4
        h0 = (c % 4) * HPC
        O[b, h0 : h0 + HPC] = r["o"]
    return O


# revision 16
# speedup vs baseline: 1.6426x; 1.0353x over previous
"""MQA attention kernel for Trainium2, 8-core SPMD (v5).

Problem: Q [2, 8, 2048, 64] fp32, K/V [2, 1, 2048, 64] fp32 (shared head).
out[b,h,q,:] = softmax(Q[b,h,q,:] @ K[b,0]^T / 8) @ V[b,0].

Sharding: 16 (b,h) pairs over 8 cores -> core c handles b = c//4,
heads 2*(c%4), 2*(c%4)+1 (both heads share one K/V slice).

Design notes:
  - "Permuted-s" staging: all inputs load as Xn[p, c, ...] = X[s=16p+c]
    (128 x 4KB contiguous descriptors per DMA, ~20x fewer than a
    128-partition-tile layout). Queries and keys are processed in this
    permuted order everywhere on-chip (softmax is order-invariant; V uses
    the same key order), and only the final output DMA un-permutes.
  - Zero PE transposes / zero SWDGE / zero DRAM scratch: fp32 HWDGE
    loads, fp16 casts on ACT+DVE (idle in the prologue), QT/KT [128, S]
    via merged SBUF->SBUF XBAR transpose-DMAs (3D-out = chunked
    partition transposes; K is cast column-duplicated so K^T lands
    replicated on both partition halves for free).
  - MM1 (scores^T): per k-tile, two concurrent row-group matmuls
    (contract=64; h0 rows 0-63, h1 rows 64-127) -> ps_s [128k, 2, 512q].
  - exp split: ACT true exp for h0; DVE Schraudolph exp for h1 in one
    tensor_scalar: int16(round(score*A+B)) bitcast fp16 ~= exp(score/8-C)
    (the -C shift cancels in softmax).
  - MM2: out^T[128, 512] += V_aug[kt]^T @ P^T; V_aug = [V | ones | 0-pad
    to 128 cols]. Col 64 accumulates the softmax denominator.
  - Drain (per qb, h): PSUM->SBUF fp16 copy rows 0:80 (ACT h0 / DVE h1),
    ONE merged XBAR transpose [80,512]->[128,4,80], DVE reciprocal of
    col 64, DVE per-chunk tensor_scalar normalize -> fp32, DMA out.
    Drain is EMITTED INTERLEAVED into the next qb's kt loop; all drain
    pools are 4-deep so a slow stage never backpressures the loop.
  - Main loop software-pipelined with MM2 trailing MM1 by 3 k-tiles so
    the PE FIFO never waits on exp; ps_s/ps_o double-buffered = 8 banks.
  - Two PE warmup phases bridge the staging window so HAM is at K=8/8
    when the main loop starts.
"""

import numpy as np

import concourse.bass as bass
import concourse.bacc as bacc
import concourse.mybir as mybir
import concourse.tile as tile
from concourse.bass_utils import run_bass_kernel_spmd

F32 = mybir.dt.float32
F16 = mybir.dt.float16
I16 = mybir.dt.int16

B, H, S, D = 2, 8, 2048, 64
HPC = 2            # heads per core
NCORES = 8
QB = 512           # query block (PSUM bank free-dim limit for fp32)
NQB = S // QB      # 4
KT_TILE = 128      # keys per k-tile (PE contract partition limit)
NKT = S // KT_TILE # 16
NC = NKT // NQB    # 4 c-chunks per qb block
MO = 80            # drained rows: 64 V + 1 denom + 15 pad (16 | 80)
VW = 128           # V_aug weight cols
SCALE = 1.0 / np.sqrt(np.float32(D))  # 0.125
C_SHIFT = 2.0      # exp(z - C_SHIFT): bounds p in fp16/int16; cancels in softmax

# Schraudolph fp16 exp: i16 = round(score*A2 + B2); bitcast fp16 ~= exp(score/8 - C)
A_FP16 = 1024.0 / np.log(2.0)              # 2^10 / ln 2
A2 = float(SCALE) * A_FP16                 # folded score scale
B2 = 15.0 * 1024.0 - C_SHIFT * A_FP16      # exponent bias - shift
MM2_SKEW = 2       # effective skew = MM2_SKEW + 1


def build_nc():
    nc = bacc.Bacc(None)
    Qd = nc.declare_dram_parameter("q", [HPC, S, D], F32, isOutput=False)
    Kd = nc.declare_dram_parameter("k", [S, D], F32, isOutput=False)
    Vd = nc.declare_dram_parameter("v", [S, D], F32, isOutput=False)
    Od = nc.declare_dram_parameter("o", [HPC, S, D], F32, isOutput=True)

    with tile.TileContext(nc) as tc:
        with (
            tc.tile_pool(name="const", bufs=1) as constp,
            tc.tile_pool(name="stage", bufs=1) as stp,
            tc.tile_pool(name="qk", bufs=1) as qkp,
            tc.tile_pool(name="vt", bufs=1) as vp,
            tc.tile_pool(name="pt", bufs=6) as ptp,
            tc.tile_pool(name="otF", bufs=2) as otFp,
            tc.tile_pool(name="otT", bufs=4) as otTp,
            tc.tile_pool(name="rec", bufs=4) as recp,
            tc.tile_pool(name="outsb", bufs=4) as outp,
            tc.tile_pool(name="psS", bufs=2, space="PSUM") as psSp,
            tc.tile_pool(name="psO", bufs=2, space="PSUM") as psOp,
        ):
            # Prime the exp table load so the ~2.7us ACT_TABLE_LOAD overlaps
            # the input DMA phase instead of stalling the first real exp.
            dummy = constp.tile([128, 64], F16)
            nc.vector.memset(dummy[:], 0.0)
            dummy32 = constp.tile([128, 16], F32)
            nc.vector.memset(dummy32[:], 0.0)
            nc.scalar.activation(
                dummy32[:], dummy32[:], mybir.ActivationFunctionType.Exp
            )

            # per-partition bias AP for the ACT exp (const-AP registry only
            # carries pre-registered values)
            bias_ap = constp.tile([128, 1], F32)
            nc.vector.memset(bias_ap[:], -float(C_SHIFT))

            # warmup output scribbles into qb0's accumulator (cleared by the
            # start=True MM2 later)
            ps_o0 = [psOp.tile([128, QB], F32, name=f"psO{h}") for h in range(HPC)]

            # PE warmup phase 1: unblocked, runs as soon as the PE is free.
            for _ in range(68):
                nc.tensor.matmul(
                    ps_o0[0][0:64, 0:64],
                    lhsT=dummy[:, 0:64],
                    rhs=dummy[:],
                    start=True,
                    stop=True,
                )

            # ---- input staging: contiguous permuted-s loads, chunked fp16
            # casts, interleaved merged XBAR transposes ----
            Kn = stp.tile([128, NKT, D], F32, name="Kn")
            nc.sync.dma_start(
                out=Kn[:], in_=Kd.ap().rearrange("(p c) d -> p c d", p=128)
            )
            Qn = stp.tile([128, NKT, HPC, D], F32, name="Qn")
            for h in range(HPC):
                nc.sync.dma_start(
                    out=Qn[:, :, h, :],
                    in_=Qd.ap()[h].rearrange("(p c) d -> p c d", p=128),
                )
            Vn = stp.tile([128, NKT, D], F32, name="Vn")
            nc.sync.dma_start(
                out=Vn[:], in_=Vd.ap().rearrange("(p c) d -> p c d", p=128)
            )

            # K cast duplicated into both 64-col halves (the XBAR consumes
            # 128-wide input column tiles; the duplicate lands K^T on both
            # partition halves of KT).
            Kh = stp.tile([128, NKT, 2, D], F16, name="Kh")
            Qh = stp.tile([128, NKT, HPC, D], F16, name="Qh")
            for c in range(NQB):
                cs = slice(c * NC, (c + 1) * NC)
                nc.vector.tensor_copy(Kh[:, cs, 0, :], Kn[:, cs, :])
                nc.vector.tensor_copy(Kh[:, cs, 1, :], Kn[:, cs, :])
                nc.scalar.copy(Qh[:, cs, :, :], Qn[:, cs, :, :])

            # V_aug [128k, c, 128] fp16: cols 0-63 = V (same permuted key
            # order as KT), col 64 = 1.0 (denominator), cols 65-127 = 0.
            Vt = vp.tile([128, NKT, VW], F16)
            nc.vector.memset(Vt[:, :, D + 1 : VW], 0.0)
            nc.vector.memset(Vt[:, :, D : D + 1], 1.0)
            nc.scalar.copy(Vt[:, :, 0:D], Vn[:])

            # PE warmup phase 2: gated on the first Q cast so it bridges the
            # cast/XBAR window right up to the main loop.
            for _ in range(40):
                nc.tensor.matmul(
                    ps_o0[0][0:64, 0:128],
                    lhsT=dummy[:, 0:64],
                    rhs=Qh[:, 0, :, :].rearrange("p h d -> p (h d)"),
                    start=True,
                    stop=True,
                )

            # KT/QT [128, c, 128] fp16 via interleaved chunked XBAR
            # transposes: col (c, p) of KT/QT is key/query s = 16p + c.
            KT = qkp.tile([128, NKT, 128], F16, name="KT")
            QT = qkp.tile([128, NKT, 128], F16, name="QT")
            for c in range(NQB):
                cs = slice(c * NC, (c + 1) * NC)
                nc.sync.dma_start(
                    out=KT[:, cs, :],
                    in_=Kh[:, cs, :, :].rearrange("p c r d -> p (c r d)"),
                    transpose=True,
                )
                nc.sync.dma_start(
                    out=QT[:, cs, :],
                    in_=Qh[:, cs, :, :].rearrange("p c h d -> p (c h d)"),
                    transpose=True,
                )

            def QTs(h, qb):
                # [64, 512] moving operand for head h, query block qb
                return (
                    QT[64 * h : 64 * (h + 1), :, :]
                    .rearrange("p c s -> p (c s)")[:, qb * QB : (qb + 1) * QB]
                )

            def KTs(h, kt):
                return KT[64 * h : 64 * (h + 1), kt, :]

            # ---- main loop; the previous qb's drain is emitted interleaved
            # into this qb's kt loop ----
            def drain_stages(qb, ps_o):
                st = {"otF": [None, None], "otT": [None, None]}

                def s_copy0():
                    st["otF"][0] = otFp.tile([MO, QB], F16, name="otF0")
                    nc.scalar.copy(st["otF"][0][:], ps_o[0][0:MO, :])

                def s_copy1():
                    st["otF"][1] = otFp.tile([MO, QB], F16, name="otF1")
                    nc.vector.tensor_copy(st["otF"][1][:], ps_o[1][0:MO, :])

                def s_transp(h):
                    def f():
                        st["otT"][h] = otTp.tile(
                            [128, NC, MO], F16, name=f"otT{h}"
                        )
                        nc.sync.dma_start(
                            out=st["otT"][h][:], in_=st["otF"][h][:], transpose=True
                        )
                    return f

                def s_norm(h):
                    def f():
                        otT = st["otT"][h]
                        rec = recp.tile([128, NC, 1], F32)
                        nc.vector.reciprocal(rec[:], otT[:, :, D : D + 1])
                        outsb = outp.tile([128, NC, D], F32)
                        for j in range(NC):
                            nc.vector.tensor_scalar_mul(
                                outsb[:, j, :], otT[:, j, 0:D], rec[:, j, :]
                            )
                        # un-permute: otT row pp, chunk j -> q = 16*pp + 4*qb + j
                        nc.sync.dma_start(
                            out=Od.ap()[h]
                            .rearrange("(p c) d -> p c d", p=128)[
                                :, qb * NC : (qb + 1) * NC, :
                            ],
                            in_=outsb[:],
                        )
                    return f

                return [s_copy0, s_copy1, s_transp(0), s_transp(1), s_norm(0), s_norm(1)]

            pending_drain = []
            for qb in range(NQB):
                ps_o = (
                    ps_o0
                    if qb == 0
                    else [psOp.tile([128, QB], F32, name=f"psO{h}") for h in range(HPC)]
                )
                pend = []  # software pipeline: MM2 trails MM1 by MM2_SKEW+1 kts
                for kt in range(NKT + MM2_SKEW + 1):
                    if kt < NKT:
                        ps_s = psSp.tile([128, HPC, QB], F32)
                        for h in range(HPC):
                            nc.tensor.matmul(
                                ps_s[:, h, :],
                                lhsT=KTs(h, kt),
                                rhs=QTs(h, qb),
                                start=True,
                                stop=True,
                            )
                    if len(pend) > (MM2_SKEW if kt < NKT else 0):
                        pkt, p0, p1 = pend.pop(0)
                        for h, rhs in ((0, p0[:]), (1, p1[:].bitcast(F16))):
                            nc.tensor.matmul(
                                ps_o[h][:],
                                lhsT=Vt[:, pkt, :],
                                rhs=rhs,
                                start=(pkt == 0),
                                stop=(pkt == NKT - 1),
                            )
                    if pending_drain and kt >= 1:
                        pending_drain.pop(0)()
                    if kt < NKT:
                        # exp: ACT (true) for h0, DVE (Schraudolph) for h1
                        pt0 = ptp.tile([128, QB], F16, name="pt0")
                        nc.scalar.activation(
                            pt0[:],
                            ps_s[:, 0, :],
                            mybir.ActivationFunctionType.Exp,
                            scale=float(SCALE),
                            bias=bias_ap[:],
                        )
                        pt1 = ptp.tile([128, QB], I16, name="pt1")
                        nc.vector.tensor_scalar(
                            pt1[:],
                            ps_s[:, 1, :],
                            float(A2),
                            float(B2),
                            op0=mybir.AluOpType.mult,
                            op1=mybir.AluOpType.add,
                        )
                        pend.append((kt, pt0, pt1))
                assert not pend and not pending_drain
                pending_drain = drain_stages(qb, ps_o)
            for f in pending_drain:
                f()
    nc.compile()
    return nc


_CACHED = {}


def _get_nc():
    if "nc" not in _CACHED:
        _CACHED["nc"] = build_nc()
    return _CACHED["nc"]


def _shard(Q, K, V):
    in_maps = []
    for c in range(NCORES):
        b = c // 4
        h0 = (c % 4) * HPC
        in_maps.append(
            {
                "q": np.ascontiguousarray(np.asarray(Q, np.float32)[b, h0 : h0 + HPC]),
                "k": np.ascontiguousarray(np.asarray(K, np.float32)[b, 0]),
                "v": np.ascontiguousarray(np.asarray(V, np.float32)[b, 0]),
            }
        )
    return in_maps


def kernel(Q, K, V, trace=False):
    nc = _get_nc()
    res = run_bass_kernel_spmd(nc, _shard(Q, K, V), list(range(NCORES)), trace=trace)
    _CACHED["last_result"] = res
    O = np.empty((B, H, S, D), np.float32)
    for c, r in enumerate(res.results):
        b = c // 4
        h0 = (c % 4) * HPC
        O[b, h0 : h0 + HPC] = r["o"]
    return O
